# revision 20
# baseline (speedup 1.0000x reference)
"""Trainium2 Bass kernel for nn_ReLULocalZeroToken (gated token-wise FFN).

Computes: w = sigmoid(x @ Wg + bg); mask = (w >= 0.5) | (labels == -100)
          y = relu((x*w) @ W1 + b1) @ W2 + b2;  out = mask ? y : 0

Sharding: data-parallel over the 16384 tokens across 8 NeuronCores
(2048 tokens/core); FFN weights replicated (bf16) on every core.
"""
import sys
if '/opt/trn_rl_repo' not in sys.path:
    sys.path.insert(0, '/opt/trn_rl_repo')

import numpy as np
import ml_dtypes

import concourse.bass as bass
import concourse.mybir as mybir
from concourse.bass_utils import run_bass_kernel_spmd

F32 = mybir.dt.float32
BF16 = mybir.dt.bfloat16

B, S, H, DFF = 2, 8192, 1024, 4096
N_CORES = 8
T_TOTAL = B * S              # 16384 tokens
T_CORE = T_TOTAL // N_CORES  # 2048 tokens per core

KH = H // 128                # 8 k-tiles over H
ND = DFF // 128              # 32 dff tiles
NPAIR = ND // 2              # 16 dff pairs (2 tiles share one PSUM bank)

T_CHUNK = 256                # tokens per pipeline chunk
NCHUNK = T_CORE // T_CHUNK   # 8
NGRP = T_CHUNK // 128        # 2 groups of 128 tokens per chunk
N_CONST = 8                  # number of constant-upload DMAs


def _bcast(ap, p=128):
    """DRAM AP broadcast across p partitions (0-stride partition dim)."""
    return bass.AP(tensor=ap.tensor, offset=ap.offset, ap=[[0, p], *ap.ap])


def build_kernel(t_core=T_CORE, stage=4, relu_mode='both', repeat=1):
    # stage: pipeline truncation for HW bisection. 0=DMA only, 1=+gate,
    # 2=+transposes/xT, 3=+matmul1/relu, 4=full.
    nchunk = t_core // T_CHUNK
    ngall = t_core // 128  # total 128-token groups per core
    ctot = nchunk * repeat  # total pipeline chunks (repeat>1: timing builds)

    nc = bass.Bass(target_bir_lowering=False)

    x_d = nc.dram_tensor("x", [t_core, H], F32, kind="ExternalInput")
    wg_d = nc.dram_tensor("wg", [H], F32, kind="ExternalInput")
    bg_d = nc.dram_tensor("bg", [1], F32, kind="ExternalInput")
    w1_d = nc.dram_tensor("w1", [H, DFF], BF16, kind="ExternalInput")
    b1_d = nc.dram_tensor("b1", [DFF], F32, kind="ExternalInput")
    w2_d = nc.dram_tensor("w2", [DFF, H], BF16, kind="ExternalInput")
    b2_d = nc.dram_tensor("b2", [H], F32, kind="ExternalInput")
    lab_d = nc.dram_tensor("lab", [t_core], F32, kind="ExternalInput")
    id_d = nc.dram_tensor("ident", [128, 128], F32, kind="ExternalInput")
    out_d = nc.dram_tensor("out", [t_core, H], F32, kind="ExternalOutput")

    x_v = x_d.rearrange("(c g p) h -> c p g h", p=128, g=NGRP)   # [nchunk,128,2,1024]
    out_v = out_d.rearrange("(g p) h -> g p h", p=128)           # [ngall,128,1024]
    w1_v = w1_d.rearrange("(k p) m -> p k m", p=128)             # [128,8,4096]
    w2_v = w2_d.rearrange("(k p) n -> p k n", p=128)             # [128,32,1024]
    # b1/lab arrive host-prepacked partition-major: flat[(p, d)] = orig[d*128+p]
    b1_v = b1_d.rearrange("(p d) -> p d", p=128)                 # [128,32]
    lab_v = lab_d.rearrange("(p g) -> p g", p=128)               # [128,ngall]

    import contextlib
    ctx = contextlib.ExitStack()
    with ctx:
        # ---- SBUF ----
        w1_sb = ctx.enter_context(nc.sbuf_tensor("w1_sb", [128, KH, DFF], BF16))
        w2_sb = ctx.enter_context(nc.sbuf_tensor("w2_sb", [128, ND, H], BF16))
        wg_b = ctx.enter_context(nc.sbuf_tensor("wg_b", [128, H], F32))
        b1_sb = ctx.enter_context(nc.sbuf_tensor("b1_sb", [128, ND], F32))
        b2_b = ctx.enter_context(nc.sbuf_tensor("b2_b", [128, H], F32))
        bg_sb = ctx.enter_context(nc.sbuf_tensor("bg_sb", [128, 1], F32))
        lab_sb = ctx.enter_context(nc.sbuf_tensor("lab_sb", [128, ngall], F32))
        ign_sb = ctx.enter_context(nc.sbuf_tensor("ign_sb", [128, ngall], F32))
        id_sb = ctx.enter_context(nc.sbuf_tensor("id_sb", [128, 128], F32))
        logit_sb = ctx.enter_context(nc.sbuf_tensor("logit_sb", [128, ngall], F32))
        wgt_sb = ctx.enter_context(nc.sbuf_tensor("wgt_sb", [128, ngall], F32))
        mask_sb = ctx.enter_context(nc.sbuf_tensor("mask_sb", [128, ngall], F32))
        prod_sb = ctx.enter_context(nc.sbuf_tensor("prod_sb", [128, H], F32))
        xt0 = ctx.enter_context(nc.sbuf_tensor("xt0", [128, NGRP, H], F32))
        xt1 = ctx.enter_context(nc.sbuf_tensor("xt1", [128, NGRP, H], F32))
        xT0 = ctx.enter_context(nc.sbuf_tensor("xT0", [128, KH, T_CHUNK], BF16))
        xT1 = ctx.enter_context(nc.sbuf_tensor("xT1", [128, KH, T_CHUNK], BF16))
        h1T0 = ctx.enter_context(nc.sbuf_tensor("h1T0", [128, ND, T_CHUNK], BF16))
        h1T1 = ctx.enter_context(nc.sbuf_tensor("h1T1", [128, ND, T_CHUNK], BF16))
        ysb0 = ctx.enter_context(nc.sbuf_tensor("ysb0", [128, H], F32))
        ysb1 = ctx.enter_context(nc.sbuf_tensor("ysb1", [128, H], F32))
        x_tile = [xt0, xt1]
        xT = [xT0, xT1]
        h1T = [h1T0, h1T1]
        y_sb = [ysb0, ysb1]

        # ---- PSUM (8 banks total) ----
        pt0 = ctx.enter_context(nc.psum_tensor("pt0", [128, 512], F32))
        pt1 = ctx.enter_context(nc.psum_tensor("pt1", [128, 512], F32))
        ph0 = ctx.enter_context(nc.psum_tensor("ph0", [128, 512], F32))
        ph1 = ctx.enter_context(nc.psum_tensor("ph1", [128, 512], F32))
        py0 = ctx.enter_context(nc.psum_tensor("py0", [128, 1024], F32))
        py1 = ctx.enter_context(nc.psum_tensor("py1", [128, 1024], F32))
        pt = [pt0, pt1]
        ph = [ph0, ph1]
        py = [py0, py1]

        # ---- semaphores ----
        sem = {}
        for name in ["s_wc", "s_ww", "s_x0", "s_x1", "s_logit", "s_sig",
                     "s_gate", "s_tp0", "s_tp1", "s_tpc0", "s_tpc1",
                     "s_mm1_0", "s_mm1_1", "s_h1a0", "s_h1a1", "s_h1b0",
                     "s_h1b1", "s_mm2_0", "s_mm2_1", "s_ep0", "s_ep1",
                     "s_yout0", "s_yout1", "s_ysb0", "s_ysb1"]:
            sem[name] = ctx.enter_context(nc.semaphore(name))
        s_wc = sem["s_wc"]; s_ww = sem["s_ww"]
        s_logit = sem["s_logit"]; s_sig = sem["s_sig"]; s_gate = sem["s_gate"]
        s_x = [sem["s_x0"], sem["s_x1"]]
        s_ysb = [sem["s_ysb0"], sem["s_ysb1"]]
        s_tp = [sem["s_tp0"], sem["s_tp1"]]
        s_tpc = [sem["s_tpc0"], sem["s_tpc1"]]
        s_mm1 = [sem["s_mm1_0"], sem["s_mm1_1"]]
        s_h1a = [sem["s_h1a0"], sem["s_h1a1"]]
        s_h1b = [sem["s_h1b0"], sem["s_h1b1"]]
        s_mm2 = [sem["s_mm2_0"], sem["s_mm2_1"]]
        s_ep = [sem["s_ep0"], sem["s_ep1"]]
        s_yout = [sem["s_yout0"], sem["s_yout1"]]

        blk = ctx.enter_context(nc.Block())

        # ================= SP: all DMA =================
        @blk.sync
        def _(sync):
            sync.dma_start(wg_b[:], _bcast(wg_d[:])).then_inc(s_wc, 16)
            sync.dma_start(b1_sb[:], b1_v[:]).then_inc(s_wc, 16)
            sync.dma_start(lab_sb[:], lab_v[:]).then_inc(s_wc, 16)
            sync.dma_start(b2_b[:], _bcast(b2_d[:])).then_inc(s_wc, 16)
            sync.dma_start(bg_sb[:], _bcast(bg_d[:])).then_inc(s_wc, 16)
            sync.dma_start(id_sb[:], id_d[:]).then_inc(s_wc, 16)

            # prime first two chunk loads before the big weight DMAs so the
            # gate/transpose front of the pipeline starts immediately
            for c in range(min(2, ctot)):
                sync.dma_start(x_tile[c % 2][:], x_v[c % nchunk]).then_inc(
                    s_x[c % 2], 16)
            sync.dma_start(w1_sb[:], w1_v[:]).then_inc(s_ww, 16)
            sync.dma_start(w2_sb[:], w2_v[:]).then_inc(s_ww, 16)

            for c in range(ctot):
                if c + 2 < ctot:
                    # x_tile[c%2] free once PE transposes (s_tp1 covers all,
                    # PE retires in order) and DVE scales of chunk c are done
                    if stage >= 2:
                        sync.wait_ge(s_tp[1], 2 * (c + 1))
                    if stage >= 1:
                        sync.wait_ge(s_gate, NGRP * (c + 1))
                    if stage < 4:
                        sync.wait_ge(s_ysb[0], c + 1)
                        sync.wait_ge(s_ysb[1], c + 1)
                    sync.dma_start(x_tile[c % 2][:],
                                   x_v[(c + 2) % nchunk]).then_inc(
                        s_x[c % 2], 16)
                for m in range(NGRP):
                    u = NGRP * c + m
                    slot = u % 2
                    sync.wait_ge(s_ysb[slot], u // 2 + 1)
                    sync.dma_start(out_v[u % ngall], y_sb[slot][:]).then_inc(
                        s_yout[slot], 16)

        # ================= DVE =================
        @blk.vector
        def _(vector):
            vector.wait_ge(s_wc, 96)
            # ignored-label mask (labels == -100), once
            vector.tensor_scalar(out=ign_sb[:], in0=lab_sb[:], scalar1=-100.0,
                                 scalar2=None, op0=mybir.AluOpType.is_equal)
            vector.drain()

            def gate(cc):
                """logits for chunk cc; mask; (after sigmoid) scale x in place."""
                vector.wait_ge(s_x[cc % 2], 16 * (cc // 2 + 1))
                for g in range(NGRP):
                    col = (NGRP * cc + g) % ngall
                    vector.tensor_tensor(out=prod_sb[:],
                                         in0=x_tile[cc % 2][:, g, :],
                                         in1=wg_b[:], op=mybir.AluOpType.mult)
                    vector.drain()
                    ins = vector.tensor_reduce(out=logit_sb[:, col:col + 1],
                                               in_=prod_sb[:],
                                               axis=mybir.AxisListType.X,
                                               op=mybir.AluOpType.add)
                    vector.drain()
                    if g == NGRP - 1:
                        ins.then_inc(s_logit, 1)
                c0 = (NGRP * cc) % ngall
                # keep = (logit + bg) >= 0 ; mask = max(keep, ignored)
                vector.tensor_scalar(out=mask_sb[:, c0:c0 + NGRP],
                                     in0=logit_sb[:, c0:c0 + NGRP],
                                     scalar1=bg_sb[:, 0:1], scalar2=0.0,
                                     op0=mybir.AluOpType.add,
                                     op1=mybir.AluOpType.is_ge)
                vector.drain()
                vector.tensor_tensor(out=mask_sb[:, c0:c0 + NGRP],
                                     in0=mask_sb[:, c0:c0 + NGRP],
                                     in1=ign_sb[:, c0:c0 + NGRP],
                                     op=mybir.AluOpType.max)
                vector.drain()
                vector.wait_ge(s_sig, cc + 1)
                for g in range(NGRP):
                    col = (NGRP * cc + g) % ngall
                    vector.tensor_scalar_mul(
                        out=x_tile[cc % 2][:, g, :],
                        in0=x_tile[cc % 2][:, g, :],
                        scalar1=wgt_sb[:, col:col + 1]).then_inc(s_gate, 1)

            if stage >= 1:
                gate(0)

            for c in range(ctot):
                # xT copies: psum transposes -> bf16 SBUF
                for hg in range(2 * NGRP) if stage >= 2 else []:
                    u = 2 * NGRP * c + hg
                    slot = u % 2
                    g, kh = hg // 2, hg % 2
                    vector.wait_ge(s_tp[slot], u // 2 + 1)
                    src = pt[slot][:, 0:512].rearrange("p (j t) -> p j t", j=4)
                    dst = xT[c % 2][:, 4 * kh:4 * kh + 4, 128 * g:128 * (g + 1)]
                    vector.tensor_copy(out=dst, in_=src).then_inc(s_tpc[slot], 1)
                # gate for next chunk (overlaps PE mm1 of this chunk)
                if stage >= 1 and c + 1 < ctot:
                    gate(c + 1)
                if stage < 1:
                    vector.wait_ge(s_x[c % 2], 16 * (c // 2 + 1))
                # relu: DVE consumes slot-1 ph banks entirely (both
                # halves) -- a PSUM bank is single-ported, so exactly one
                # engine may touch a bank at a time
                for pi in range(NPAIR) if stage >= 3 else []:
                    u = NPAIR * c + pi
                    if u % 2 != 1:
                        continue
                    vector.wait_ge(s_mm1[1], u // 2 + 1)
                    for half in range(2):
                        d = 2 * pi + half
                        ins = vector.tensor_scalar(
                            out=h1T[c % 2][:, d, :],
                            in0=ph[1][:, 256 * half:256 * (half + 1)],
                            scalar1=b1_sb[:, d:d + 1], scalar2=0.0,
                            op0=mybir.AluOpType.add,
                            op1=mybir.AluOpType.max)
                        if half == 1:
                            ins.then_inc(s_h1b[1], 1)
                # epilogue: y = (psum + b2) * mask ; write to y_sb
                if stage < 4:
                    for m in range(NGRP):
                        u = NGRP * c + m
                        slot = u % 2
                        if u >= 2:
                            vector.wait_ge(s_yout[slot], 16 * (u // 2))
                        vector.tensor_copy(out=y_sb[slot][:],
                                           in_=x_tile[c % 2][:, m, :]
                                           ).then_inc(s_ysb[slot], 1)
                    continue
                for m in range(NGRP):
                    u = NGRP * c + m
                    slot = u % 2
                    vector.wait_ge(s_mm2[slot], u // 2 + 1)
                    if u >= 2:
                        vector.wait_ge(s_yout[slot], 16 * (u // 2))
                    vector.tensor_tensor(out=y_sb[slot][:], in0=py[slot][:],
                                         in1=b2_b[:], op=mybir.AluOpType.add
                                         ).then_inc(s_ep[slot], 1)
                    vector.drain()
                    vector.tensor_scalar_mul(out=y_sb[slot][:],
                                             in0=y_sb[slot][:],
                                             scalar1=mask_sb[:, u % ngall:u % ngall + 1]
                                             ).then_inc(s_ysb[slot], 1)

        # ================= ACT =================
        @blk.scalar
        def _(scalar):
            scalar.wait_ge(s_wc, 96)

            def sigmoid(cc):
                c0 = (NGRP * cc) % ngall
                scalar.wait_ge(s_logit, cc + 1)
                scalar.activation(out=wgt_sb[:, c0:c0 + NGRP],
                                  in_=logit_sb[:, c0:c0 + NGRP],
                                  func=mybir.ActivationFunctionType.Sigmoid,
                                  bias=bg_sb[:, 0:1], scale=1.0
                                  ).then_inc(s_sig, 1)

            if stage >= 1:
                sigmoid(0)
            for c in range(ctot):
                if stage >= 1 and c + 1 < ctot:
                    sigmoid(c + 1)
                # relu: ACT consumes slot-0 ph banks entirely
                for pi in range(NPAIR) if stage >= 3 else []:
                    u = NPAIR * c + pi
                    if u % 2 != 0:
                        continue
                    scalar.wait_ge(s_mm1[0], u // 2 + 1)
                    for half in range(2):
                        d = 2 * pi + half
                        ins = scalar.activation(
                            out=h1T[c % 2][:, d, :],
                            in_=ph[0][:, 256 * half:256 * (half + 1)],
                            func=mybir.ActivationFunctionType.Relu,
                            bias=b1_sb[:, d:d + 1], scale=1.0)
                        if half == 1:
                            ins.then_inc(s_h1a[0], 1)

        # ================= PE =================
        @blk.tensor
        def _(tensor):
            if stage < 2:
                return
            tensor.wait_ge(s_wc, 96)
            first_mm = True
            for c in range(ctot):
                tensor.wait_ge(s_gate, NGRP * (c + 1))
                # transposes: x (f32, token-major) -> pt psum (h-major)
                for hg in range(2 * NGRP):
                    u = 2 * NGRP * c + hg
                    slot = u % 2
                    g, kh = hg // 2, hg % 2
                    if u >= 2:
                        tensor.wait_ge(s_tpc[slot], u // 2)
                    for j in range(4):
                        k = 4 * kh + j
                        ins = tensor.transpose(
                            pt[slot][:, 128 * j:128 * (j + 1)],
                            x_tile[c % 2][:, g, 128 * k:128 * (k + 1)],
                            id_sb[:])
                        if j == 3:
                            ins.then_inc(s_tp[slot], 1)
                # wait until DVE finished all xT copies for this chunk
                tensor.wait_ge(s_tpc[0], 2 * c + 2)
                tensor.wait_ge(s_tpc[1], 2 * c + 2)
                if stage < 3:
                    continue
                if first_mm:
                    tensor.wait_ge(s_ww, 32)
                    first_mm = False
                # matmul1: h1T[d] = W1[:,d].T @ xT   (accumulate over KH)
                for pi in range(NPAIR):
                    u = NPAIR * c + pi
                    slot = u % 2
                    if u >= 2:
                        if slot == 0:
                            tensor.wait_ge(s_h1a[0], u // 2)
                        else:
                            tensor.wait_ge(s_h1b[1], u // 2)
                    for half in range(2):
                        d = 2 * pi + half
                        for k in range(KH):
                            ins = tensor.matmul(
                                ph[slot][:, 256 * half:256 * (half + 1)],
                                w1_sb[:, k, 128 * d:128 * (d + 1)],
                                xT[c % 2][:, k, :],
                                start=(k == 0), stop=(k == KH - 1))
                            if half == 1 and k == KH - 1:
                                ins.then_inc(s_mm1[slot], 1)
                if stage < 4:
                    continue
                # all relu writes for this chunk must land before matmul2
                tensor.wait_ge(s_h1a[0], NPAIR // 2 * (c + 1))
                tensor.wait_ge(s_h1b[1], NPAIR // 2 * (c + 1))
                # matmul2: y[m] = h1T[:, :, m].T @ W2  (accumulate over ND)
                for m in range(NGRP):
                    u = NGRP * c + m
                    slot = u % 2
                    if u >= 2:
                        tensor.wait_ge(s_ep[slot], u // 2)
                    for nh in range(2):
                        for d in range(ND):
                            ins = tensor.matmul(
                                py[slot][:, 512 * nh:512 * (nh + 1)],
                                h1T[c % 2][:, d, 128 * m:128 * (m + 1)],
                                w2_sb[:, d, 512 * nh:512 * (nh + 1)],
                                start=(d == 0), stop=(d == ND - 1))
                            if nh == 1 and d == ND - 1:
                                ins.then_inc(s_mm2[slot], 1)

    return nc


_NC_CACHE = {}


def _get_nc(t_core=T_CORE):
    if t_core not in _NC_CACHE:
        _NC_CACHE[t_core] = build_kernel(t_core)
    return _NC_CACHE[t_core]


def kernel(hidden_states, cos, sin, Wg, bg, W1, b1, W2, b2, labels,
           cu_seq_lens_q, **_unused):
    x = np.ascontiguousarray(np.asarray(hidden_states, dtype=np.float32)
                             ).reshape(T_TOTAL, H)
    lab = np.ascontiguousarray(np.asarray(labels)).reshape(T_TOTAL)
    lab_f = lab.astype(np.float32)
    w1_bf = np.asarray(W1, dtype=np.float32).astype(ml_dtypes.bfloat16)
    w2_bf = np.asarray(W2, dtype=np.float32).astype(ml_dtypes.bfloat16)
    wg_f = np.ascontiguousarray(np.asarray(Wg, dtype=np.float32)).reshape(H)
    bg_f = np.ascontiguousarray(np.asarray(bg, dtype=np.float32)).reshape(1)
    b1_f = np.ascontiguousarray(np.asarray(b1, dtype=np.float32)).reshape(DFF)
    b2_f = np.ascontiguousarray(np.asarray(b2, dtype=np.float32)).reshape(H)
    ident = np.eye(128, dtype=np.float32)

    nc = _get_nc()
    b1_packed = np.ascontiguousarray(b1_f.reshape(ND, 128).T).reshape(DFF)
    in_maps = []
    for r in range(N_CORES):
        sl = slice(r * T_CORE, (r + 1) * T_CORE)
        lab_packed = np.ascontiguousarray(
            lab_f[sl].reshape(-1, 128).T).reshape(T_CORE)
        in_maps.append({
            "x": np.ascontiguousarray(x[sl]),
            "wg": wg_f, "bg": bg_f,
            "w1": w1_bf, "b1": b1_packed,
            "w2": w2_bf, "b2": b2_f,
            "lab": lab_packed,
            "ident": ident,
        })
    res = run_bass_kernel_spmd(nc, in_maps, core_ids=list(range(N_CORES)))
    out = np.concatenate([res.results[r]["out"] for r in range(N_CORES)],
                         axis=0)
    return out.reshape(B, S, H)


# revision 22
# speedup vs baseline: 1.3453x; 1.3453x over previous
"""Trainium2 Bass kernel for nn_ReLULocalZeroToken (gated token-wise FFN).

Computes: w = sigmoid(x @ Wg + bg); mask = (w >= 0.5) | (labels == -100)
          y = relu((x*w) @ W1 + b1) @ W2 + b2;  out = mask ? y : 0

Sharding: data-parallel over the 16384 tokens across 8 NeuronCores
(2048 tokens/core); FFN weights replicated (bf16) on every core.
"""
import sys
if '/opt/trn_rl_repo' not in sys.path:
    sys.path.insert(0, '/opt/trn_rl_repo')

import numpy as np
import ml_dtypes

import concourse.bass as bass
import concourse.mybir as mybir
from concourse.bass_utils import run_bass_kernel_spmd

F32 = mybir.dt.float32
BF16 = mybir.dt.bfloat16

B, S, H, DFF = 2, 8192, 1024, 4096
N_CORES = 8
T_TOTAL = B * S              # 16384 tokens
T_CORE = T_TOTAL // N_CORES  # 2048 tokens per core

KH = H // 128                # 8 k-tiles over H
ND = DFF // 128              # 32 dff tiles
NPAIR = ND // 2              # 16 dff pairs (2 tiles share one PSUM bank)

T_CHUNK = 512                # tokens per pipeline chunk
NCHUNK = T_CORE // T_CHUNK   # 8
NGRP = T_CHUNK // 128        # 2 groups of 128 tokens per chunk
N_CONST = 8                  # number of constant-upload DMAs


def _bcast(ap, p=128):
    """DRAM AP broadcast across p partitions (0-stride partition dim)."""
    return bass.AP(tensor=ap.tensor, offset=ap.offset, ap=[[0, p], *ap.ap])


def build_kernel(t_core=T_CORE, stage=4, relu_mode='both', repeat=1):
    # stage: pipeline truncation for HW bisection. 0=DMA only, 1=+gate,
    # 2=+transposes/xT, 3=+matmul1/relu, 4=full.
    nchunk = t_core // T_CHUNK
    ngall = t_core // 128  # total 128-token groups per core
    ctot = nchunk * repeat  # total pipeline chunks (repeat>1: timing builds)

    nc = bass.Bass(target_bir_lowering=False)

    x_d = nc.dram_tensor("x", [t_core, H], F32, kind="ExternalInput")
    wg_d = nc.dram_tensor("wg", [H], F32, kind="ExternalInput")
    bg_d = nc.dram_tensor("bg", [1], F32, kind="ExternalInput")
    w1_d = nc.dram_tensor("w1", [H, DFF], BF16, kind="ExternalInput")
    b1_d = nc.dram_tensor("b1", [DFF], F32, kind="ExternalInput")
    w2_d = nc.dram_tensor("w2", [DFF, H], BF16, kind="ExternalInput")
    b2_d = nc.dram_tensor("b2", [H], F32, kind="ExternalInput")
    lab_d = nc.dram_tensor("lab", [t_core], F32, kind="ExternalInput")
    id_d = nc.dram_tensor("ident", [128, 128], F32, kind="ExternalInput")
    out_d = nc.dram_tensor("out", [t_core, H], F32, kind="ExternalOutput")

    x_v = x_d.rearrange("(c g p) h -> c p g h", p=128, g=NGRP)   # [nchunk,128,2,1024]
    out_v = out_d.rearrange("(g p) h -> g p h", p=128)           # [ngall,128,1024]
    w1_v = w1_d.rearrange("(k p) m -> p k m", p=128)             # [128,8,4096]
    w2_v = w2_d.rearrange("(k p) n -> p k n", p=128)             # [128,32,1024]
    # b1/lab arrive host-prepacked partition-major: flat[(p, d)] = orig[d*128+p]
    b1_v = b1_d.rearrange("(p d) -> p d", p=128)                 # [128,32]
    lab_v = lab_d.rearrange("(p g) -> p g", p=128)               # [128,ngall]

    import contextlib
    ctx = contextlib.ExitStack()
    with ctx:
        # ---- SBUF ----
        w1_sb = ctx.enter_context(nc.sbuf_tensor("w1_sb", [128, KH, DFF], BF16))
        w2_sb = ctx.enter_context(nc.sbuf_tensor("w2_sb", [128, ND, H], BF16))
        wg_b = ctx.enter_context(nc.sbuf_tensor("wg_b", [128, H], F32))
        b1_sb = ctx.enter_context(nc.sbuf_tensor("b1_sb", [128, ND], F32))
        b2_b = ctx.enter_context(nc.sbuf_tensor("b2_b", [128, H], F32))
        bg_sb = ctx.enter_context(nc.sbuf_tensor("bg_sb", [128, 1], F32))
        lab_sb = ctx.enter_context(nc.sbuf_tensor("lab_sb", [128, ngall], F32))
        ign_sb = ctx.enter_context(nc.sbuf_tensor("ign_sb", [128, ngall], F32))
        id_sb = ctx.enter_context(nc.sbuf_tensor("id_sb", [128, 128], F32))
        logit_sb = ctx.enter_context(nc.sbuf_tensor("logit_sb", [128, ngall], F32))
        wgt_sb = ctx.enter_context(nc.sbuf_tensor("wgt_sb", [128, ngall], F32))
        mask_sb = ctx.enter_context(nc.sbuf_tensor("mask_sb", [128, ngall], F32))
        prod_sb = ctx.enter_context(nc.sbuf_tensor("prod_sb", [128, H], F32))
        xt0 = ctx.enter_context(nc.sbuf_tensor("xt0", [128, NGRP, H], F32))
        xT0 = ctx.enter_context(nc.sbuf_tensor("xT0", [128, KH, T_CHUNK], BF16))
        h1T0 = ctx.enter_context(nc.sbuf_tensor("h1T0", [128, ND, T_CHUNK], BF16))
        ysb0 = ctx.enter_context(nc.sbuf_tensor("ysb0", [128, H], F32))
        ysb1 = ctx.enter_context(nc.sbuf_tensor("ysb1", [128, H], F32))
        x_tile = [xt0, xt0]          # single physical buffer
        xT = [xT0, xT0]              # single physical buffer
        h1T = [h1T0, h1T0]           # single physical buffer
        y_sb = [ysb0, ysb1]

        # ---- PSUM (8 banks total) ----
        pt0 = ctx.enter_context(nc.psum_tensor("pt0", [128, 512], F32))
        pt1 = ctx.enter_context(nc.psum_tensor("pt1", [128, 512], F32))
        ph0 = ctx.enter_context(nc.psum_tensor("ph0", [128, 512], F32))
        ph1 = ctx.enter_context(nc.psum_tensor("ph1", [128, 512], F32))
        py0 = ctx.enter_context(nc.psum_tensor("py0", [128, 1024], F32))
        py1 = ctx.enter_context(nc.psum_tensor("py1", [128, 1024], F32))
        pt = [pt0, pt1]
        ph = [ph0, ph1]
        py = [py0, py1]

        # ---- semaphores ----
        sem = {}
        for name in ["s_wc", "s_ww", "s_x0", "s_x1", "s_logit", "s_sig",
                     "s_gate", "s_tp0", "s_tp1", "s_tpc0", "s_tpc1",
                     "s_mm1_0", "s_mm1_1", "s_h1a0", "s_h1a1", "s_h1b0",
                     "s_h1b1", "s_mm2_0", "s_mm2_1", "s_ep0", "s_ep1",
                     "s_yout0", "s_yout1", "s_ysb0", "s_ysb1"]:
            sem[name] = ctx.enter_context(nc.semaphore(name))
        s_wc = sem["s_wc"]; s_ww = sem["s_ww"]
        s_logit = sem["s_logit"]; s_sig = sem["s_sig"]; s_gate = sem["s_gate"]
        s_x = [sem["s_x0"], sem["s_x1"]]
        s_ysb = [sem["s_ysb0"], sem["s_ysb1"]]
        s_tp = [sem["s_tp0"], sem["s_tp1"]]
        s_tpc = [sem["s_tpc0"], sem["s_tpc1"]]
        s_mm1 = [sem["s_mm1_0"], sem["s_mm1_1"]]
        s_h1a = [sem["s_h1a0"], sem["s_h1a1"]]
        s_h1b = [sem["s_h1b0"], sem["s_h1b1"]]
        s_mm2 = [sem["s_mm2_0"], sem["s_mm2_1"]]
        s_ep = [sem["s_ep0"], sem["s_ep1"]]
        s_yout = [sem["s_yout0"], sem["s_yout1"]]

        blk = ctx.enter_context(nc.Block())

        # ================= SP: all DMA =================
        @blk.sync
        def _(sync):
            sync.dma_start(wg_b[:], _bcast(wg_d[:])).then_inc(s_wc, 16)
            sync.dma_start(b1_sb[:], b1_v[:]).then_inc(s_wc, 16)
            sync.dma_start(lab_sb[:], lab_v[:]).then_inc(s_wc, 16)
            sync.dma_start(b2_b[:], _bcast(b2_d[:])).then_inc(s_wc, 16)
            sync.dma_start(bg_sb[:], _bcast(bg_d[:])).then_inc(s_wc, 16)
            sync.dma_start(id_sb[:], id_d[:]).then_inc(s_wc, 16)

            # prime the first chunk load before the big weight DMAs so the
            # gate/transpose front of the pipeline starts immediately
            sync.dma_start(x_tile[0][:], x_v[0]).then_inc(s_x[0], 16)
            sync.dma_start(w1_sb[:], w1_v[:]).then_inc(s_ww, 16)
            sync.dma_start(w2_sb[:], w2_v[:]).then_inc(s_ww, 16)

            for c in range(ctot):
                if c + 1 < ctot:
                    # x buffer free once PE transposes of chunk c (s_tp[1]
                    # covers all, PE retires in order) and DVE scales done
                    sync.wait_ge(s_tp[1], NGRP * (c + 1))
                    sync.wait_ge(s_gate, NGRP * (c + 1))
                    sync.dma_start(x_tile[(c + 1) % 2][:],
                                   x_v[(c + 1) % nchunk]).then_inc(
                        s_x[(c + 1) % 2], 16)
                for m in range(NGRP):
                    u = NGRP * c + m
                    slot = u % 2
                    sync.wait_ge(s_ysb[slot], u // 2 + 1)
                    sync.dma_start(out_v[u % ngall], y_sb[slot][:]).then_inc(
                        s_yout[slot], 16)

        # ================= DVE =================
        @blk.vector
        def _(vector):
            vector.wait_ge(s_wc, 96)
            # ignored-label mask (labels == -100), once
            vector.tensor_scalar(out=ign_sb[:], in0=lab_sb[:], scalar1=-100.0,
                                 scalar2=None, op0=mybir.AluOpType.is_equal)
            vector.drain()

            def gate(cc):
                """logits for chunk cc; mask; (after sigmoid) scale x in place."""
                vector.wait_ge(s_x[cc % 2], 16 * (cc // 2 + 1))
                for g in range(NGRP):
                    col = (NGRP * cc + g) % ngall
                    vector.tensor_tensor(out=prod_sb[:],
                                         in0=x_tile[cc % 2][:, g, :],
                                         in1=wg_b[:], op=mybir.AluOpType.mult)
                    vector.drain()
                    ins = vector.tensor_reduce(out=logit_sb[:, col:col + 1],
                                               in_=prod_sb[:],
                                               axis=mybir.AxisListType.X,
                                               op=mybir.AluOpType.add)
                    vector.drain()
                    if g == NGRP - 1:
                        ins.then_inc(s_logit, 1)
                c0 = (NGRP * cc) % ngall
                # keep = (logit + bg) >= 0 ; mask = max(keep, ignored)
                vector.tensor_scalar(out=mask_sb[:, c0:c0 + NGRP],
                                     in0=logit_sb[:, c0:c0 + NGRP],
                                     scalar1=bg_sb[:, 0:1], scalar2=0.0,
                                     op0=mybir.AluOpType.add,
                                     op1=mybir.AluOpType.is_ge)
                vector.drain()
                vector.tensor_tensor(out=mask_sb[:, c0:c0 + NGRP],
                                     in0=mask_sb[:, c0:c0 + NGRP],
                                     in1=ign_sb[:, c0:c0 + NGRP],
                                     op=mybir.AluOpType.max)
                vector.drain()
                vector.wait_ge(s_sig, cc + 1)
                for g in range(NGRP):
                    col = (NGRP * cc + g) % ngall
                    vector.tensor_scalar_mul(
                        out=x_tile[cc % 2][:, g, :],
                        in0=x_tile[cc % 2][:, g, :],
                        scalar1=wgt_sb[:, col:col + 1]).then_inc(s_gate, 1)

            if stage >= 1:
                gate(0)

            for c in range(ctot):
                # xT copies: psum transposes -> bf16 SBUF
                for hg in range(2 * NGRP) if stage >= 2 else []:
                    u = 2 * NGRP * c + hg
                    slot = u % 2
                    g, kh = hg // 2, hg % 2
                    vector.wait_ge(s_tp[slot], u // 2 + 1)
                    src = pt[slot][:, 0:512].rearrange("p (j t) -> p j t", j=4)
                    dst = xT[c % 2][:, 4 * kh:4 * kh + 4, 128 * g:128 * (g + 1)]
                    vector.tensor_copy(out=dst, in_=src).then_inc(s_tpc[slot], 1)
                # gate for next chunk (overlaps PE mm1 of this chunk)
                if stage >= 1 and c + 1 < ctot:
                    gate(c + 1)
                if stage < 1:
                    vector.wait_ge(s_x[c % 2], 16 * (c // 2 + 1))
                # relu: DVE consumes slot-1 (odd-d) ph banks
                for d in range(ND) if stage >= 3 else []:
                    u = ND * c + d
                    if u % 2 != 1:
                        continue
                    vector.wait_ge(s_mm1[1], u // 2 + 1)
                    vector.tensor_scalar(
                        out=h1T[c % 2][:, d, :],
                        in0=ph[1][:, 0:T_CHUNK],
                        scalar1=b1_sb[:, d:d + 1], scalar2=0.0,
                        op0=mybir.AluOpType.add,
                        op1=mybir.AluOpType.max).then_inc(s_h1b[1], 1)
                # epilogue: y = (psum + b2) * mask ; write to y_sb
                if stage < 4:
                    for m in range(NGRP):
                        u = NGRP * c + m
                        slot = u % 2
                        if u >= 2:
                            vector.wait_ge(s_yout[slot], 16 * (u // 2))
                        vector.tensor_copy(out=y_sb[slot][:],
                                           in_=x_tile[c % 2][:, m, :]
                                           ).then_inc(s_ysb[slot], 1)
                    continue
                for m in range(NGRP):
                    u = NGRP * c + m
                    slot = u % 2
                    vector.wait_ge(s_mm2[slot], u // 2 + 1)
                    if u >= 2:
                        vector.wait_ge(s_yout[slot], 16 * (u // 2))
                    vector.tensor_tensor(out=y_sb[slot][:], in0=py[slot][:],
                                         in1=b2_b[:], op=mybir.AluOpType.add
                                         ).then_inc(s_ep[slot], 1)
                    vector.drain()
                    vector.tensor_scalar_mul(out=y_sb[slot][:],
                                             in0=y_sb[slot][:],
                                             scalar1=mask_sb[:, u % ngall:u % ngall + 1]
                                             ).then_inc(s_ysb[slot], 1)

        # ================= ACT =================
        @blk.scalar
        def _(scalar):
            scalar.wait_ge(s_wc, 96)

            def sigmoid(cc):
                c0 = (NGRP * cc) % ngall
                scalar.wait_ge(s_logit, cc + 1)
                scalar.activation(out=wgt_sb[:, c0:c0 + NGRP],
                                  in_=logit_sb[:, c0:c0 + NGRP],
                                  func=mybir.ActivationFunctionType.Sigmoid,
                                  bias=bg_sb[:, 0:1], scale=1.0
                                  ).then_inc(s_sig, 1)

            if stage >= 1:
                sigmoid(0)
            for c in range(ctot):
                if stage >= 1 and c + 1 < ctot:
                    sigmoid(c + 1)
                # relu: ACT consumes slot-0 (even-d) ph banks
                for d in range(ND) if stage >= 3 else []:
                    u = ND * c + d
                    if u % 2 != 0:
                        continue
                    scalar.wait_ge(s_mm1[0], u // 2 + 1)
                    scalar.activation(
                        out=h1T[c % 2][:, d, :],
                        in_=ph[0][:, 0:T_CHUNK],
                        func=mybir.ActivationFunctionType.Relu,
                        bias=b1_sb[:, d:d + 1], scale=1.0
                        ).then_inc(s_h1a[0], 1)

        # ================= PE =================
        @blk.tensor
        def _(tensor):
            if stage < 2:
                return
            tensor.wait_ge(s_wc, 96)
            first_mm = True
            for c in range(ctot):
                tensor.wait_ge(s_gate, NGRP * (c + 1))
                # transposes: x (f32, token-major) -> pt psum (h-major)
                for hg in range(2 * NGRP):
                    u = 2 * NGRP * c + hg
                    slot = u % 2
                    g, kh = hg // 2, hg % 2
                    if u >= 2:
                        tensor.wait_ge(s_tpc[slot], u // 2)
                    for j in range(4):
                        k = 4 * kh + j
                        ins = tensor.transpose(
                            pt[slot][:, 128 * j:128 * (j + 1)],
                            x_tile[c % 2][:, g, 128 * k:128 * (k + 1)],
                            id_sb[:])
                        if j == 3:
                            ins.then_inc(s_tp[slot], 1)
                # wait until DVE finished all xT copies for this chunk
                tensor.wait_ge(s_tpc[0], NGRP * (c + 1))
                tensor.wait_ge(s_tpc[1], NGRP * (c + 1))
                if stage < 3:
                    continue
                if first_mm:
                    tensor.wait_ge(s_ww, 32)
                    first_mm = False
                # matmul1: h1T[d] = W1[:,d].T @ xT   (accumulate over KH)
                # one PSUM bank per DFF tile, N=T_CHUNK=512
                for d in range(ND):
                    u = ND * c + d
                    slot = u % 2
                    if u >= 2:
                        if slot == 0:
                            tensor.wait_ge(s_h1a[0], u // 2)
                        else:
                            tensor.wait_ge(s_h1b[1], u // 2)
                    for k in range(KH):
                        ins = tensor.matmul(
                            ph[slot][:, 0:T_CHUNK],
                            w1_sb[:, k, 128 * d:128 * (d + 1)],
                            xT[c % 2][:, k, :],
                            start=(k == 0), stop=(k == KH - 1))
                        if k == KH - 1:
                            ins.then_inc(s_mm1[slot], 1)
                if stage < 4:
                    continue
                # all relu writes for this chunk must land before matmul2
                tensor.wait_ge(s_h1a[0], ND // 2 * (c + 1))
                tensor.wait_ge(s_h1b[1], ND // 2 * (c + 1))
                # matmul2: y[m] = h1T[:, :, m].T @ W2  (accumulate over ND)
                for m in range(NGRP):
                    u = NGRP * c + m
                    slot = u % 2
                    if u >= 2:
                        tensor.wait_ge(s_ep[slot], u // 2)
                    for nh in range(2):
                        for d in range(ND):
                            ins = tensor.matmul(
                                py[slot][:, 512 * nh:512 * (nh + 1)],
                                h1T[c % 2][:, d, 128 * m:128 * (m + 1)],
                                w2_sb[:, d, 512 * nh:512 * (nh + 1)],
                                start=(d == 0), stop=(d == ND - 1))
                            if nh == 1 and d == ND - 1:
                                ins.then_inc(s_mm2[slot], 1)

    return nc


_NC_CACHE = {}


def _get_nc(t_core=T_CORE):
    if t_core not in _NC_CACHE:
        _NC_CACHE[t_core] = build_kernel(t_core)
    return _NC_CACHE[t_core]


def kernel(hidden_states, cos, sin, Wg, bg, W1, b1, W2, b2, labels,
           cu_seq_lens_q, **_unused):
    x = np.ascontiguousarray(np.asarray(hidden_states, dtype=np.float32)
                             ).reshape(T_TOTAL, H)
    lab = np.ascontiguousarray(np.asarray(labels)).reshape(T_TOTAL)
    lab_f = lab.astype(np.float32)
    w1_bf = np.asarray(W1, dtype=np.float32).astype(ml_dtypes.bfloat16)
    w2_bf = np.asarray(W2, dtype=np.float32).astype(ml_dtypes.bfloat16)
    wg_f = np.ascontiguousarray(np.asarray(Wg, dtype=np.float32)).reshape(H)
    bg_f = np.ascontiguousarray(np.asarray(bg, dtype=np.float32)).reshape(1)
    b1_f = np.ascontiguousarray(np.asarray(b1, dtype=np.float32)).reshape(DFF)
    b2_f = np.ascontiguousarray(np.asarray(b2, dtype=np.float32)).reshape(H)
    ident = np.eye(128, dtype=np.float32)

    nc = _get_nc()
    b1_packed = np.ascontiguousarray(b1_f.reshape(ND, 128).T).reshape(DFF)
    in_maps = []
    for r in range(N_CORES):
        sl = slice(r * T_CORE, (r + 1) * T_CORE)
        lab_packed = np.ascontiguousarray(
            lab_f[sl].reshape(-1, 128).T).reshape(T_CORE)
        in_maps.append({
            "x": np.ascontiguousarray(x[sl]),
            "wg": wg_f, "bg": bg_f,
            "w1": w1_bf, "b1": b1_packed,
            "w2": w2_bf, "b2": b2_f,
            "lab": lab_packed,
            "ident": ident,
        })
    res = run_bass_kernel_spmd(nc, in_maps, core_ids=list(range(N_CORES)))
    out = np.concatenate([res.results[r]["out"] for r in range(N_CORES)],
                         axis=0)
    return out.reshape(B, S, H)
